# revision 1
# baseline (speedup 1.0000x reference)
"""Trainium2 Bass kernel for ANI-1x angular terms (P=2M pairs -> (P, 32)).

Strategy: embarrassingly data-parallel over the pair axis. Each of the 8
NeuronCores gets 250k pairs (padded to 251904 = 128*1968). Output is computed
channel-major (32, Np) in bf16 per core; the host transposes/upcasts while
unsharding.

Math restructure (avoids arccos/arctan/sqrt LUTs; uses only 2 ACT table
sets -- natural_log_exp for everything but the 2 cutoff sines):
  n0 = |v0|^2, n1 = |v1|^2, dot = v0.v1
  d_j = exp(0.5*ln(n_j))
  c   = 0.95*dot*exp(-0.5*(ln n0 + ln n1))          (= 0.95*cos_angle)
  sa  = sin(angle/2) = exp(0.5*ln(0.5 - 0.5*c))     (angle = arccos(c))
  sb  = cos(angle/2) = exp(0.5*ln(0.5 + 0.5*c))
  gg_s = cos((angle - z_s)/2) = sb*cos(z_s/2) + sa*sin(z_s/2)  > 0
  factor1[s] = ((1+cos(angle-z_s))/2)^zeta = gg_s^(2*zeta) = exp(2*zeta*ln(gg_s))
  fc(d) = 0.5*cos(pi*d/3.5)+0.5 = 1 - sin^2(pi*d/7)    (sin arg stays < pi)
  fcj2 = 2*fc(d0)*fc(d1) = (sin^2(pi*d1/7) - 1) * (2*sin^2(pi*d0/7) - 2)
  factor2[a] = exp(-eta*((d0+d1)/2 - ShfA[a])^2)
  out[a*8+s] = factor1[s] * (factor2[a]*fcj2)
"""


import math
import sys

import numpy as np

try:
    import concourse.bass as bass
except ImportError:  # fresh grading dir may not have the repo on sys.path
    sys.path.insert(0, "/opt/trn_rl_repo")
    import concourse.bass as bass

import concourse.tile as tile
from concourse import bacc
from concourse import mybir
from concourse.bass_utils import run_bass_kernel_spmd

P_TOTAL = 2_000_000
N_CORES = 8
PC = P_TOTAL // N_CORES  # 250_000 pairs per core
T = 1968                 # free-dim columns per partition (128*T = padded pairs)
NP_PAD = 128 * T         # 251_904
TH = T // 2              # phase-A half and C s-loop column-half
FO = 492                 # column chunk for the final outer-product stage
N_CHUNK = T // FO

F32 = mybir.dt.float32
BF16 = mybir.dt.bfloat16

LAST_RESULT = None  # set by kernel(); test.py reads exec_time_ns from here


def _build(eta: float, zeta: float, shfa, shfz):
    A = mybir.ActivationFunctionType
    Op = mybir.AluOpType
    PI = math.pi
    se = math.sqrt(eta)

    # Uniform ShfZ spacing enables the Chebyshev-style recurrence
    # gg_{s+1} = 2*cos(dz/2)*gg_s - gg_{s-1}
    dzs = [shfz[s + 1] - shfz[s] for s in range(7)]
    uniform_z = max(dzs) - min(dzs) < 1e-5
    dz2 = (shfz[1] - shfz[0]) / 2.0 if uniform_z else None

    nc = bacc.Bacc("TRN2", target_bir_lowering=False)
    vin = nc.declare_dram_parameter("vectors12", [2, NP_PAD, 3], F32, isOutput=False)
    out = nc.declare_dram_parameter("out", [32, NP_PAD], BF16, isOutput=True)

    # Bias constants used by activation ops (bias must be a const AP in SBUF).
    bias_vals = []
    for bv in [0.5] + [-se * float(sa_) for sa_ in shfa]:
        if (F32, bv) not in nc.const_aps.aps and bv not in bias_vals:
            bias_vals.append(bv)
    const_np = np.tile(np.asarray(bias_vals, dtype=np.float32), (128, 1))
    const_dram = nc.inline_tensor(const_np, name="bias_consts")

    # HBM views: [128 partitions, ...] contiguous per partition
    v01_h = vin.rearrange("j (q t) c -> q j (t c)", q=128)  # [128, 2, 3T]
    out_h = out.rearrange("k (q t) -> q k t", q=128)

    H = 2           # column parts pipelined through phases A->B->C
    TP = T // H     # 984 columns per part
    NQ = 3          # input DMA pieces per part
    TQ = TP // NQ   # 328

    act_chain = [None]

    with tile.TileContext(nc) as tc:
        from contextlib import ExitStack
        from concourse.tile_rust import add_dep_helper

        def act(*args, **kwargs):
            return nc.scalar.activation(*args, **kwargs)

        with ExitStack() as ctx:
            pConst = ctx.enter_context(tc.tile_pool(name="pConst", bufs=1))
            ctile = pConst.tile([128, len(bias_vals)], F32, tag="consts")
            cdma = [False]

            def load_consts():
                # issued after the first input DMA so it never delays it
                nc.sync.dma_start(out=ctile[:], in_=const_dram[:])
                cdma[0] = True
            for i, bv in enumerate(bias_vals):
                nc.const_aps.aps[(F32, bv)] = ctile[:, i : i + 1]

            pP = ctx.enter_context(tc.tile_pool(name="pP", bufs=2))
            pG = ctx.enter_context(tc.tile_pool(name="pG", bufs=1))
            pV = ctx.enter_context(tc.tile_pool(name="pV", bufs=2))
            pSq = ctx.enter_context(tc.tile_pool(name="pSq", bufs=2))
            pPr = ctx.enter_context(tc.tile_pool(name="pPr", bufs=1))
            pAt = ctx.enter_context(tc.tile_pool(name="pAt", bufs=2))
            pLo = ctx.enter_context(tc.tile_pool(name="pLo", bufs=1))
            pOut = ctx.enter_context(tc.tile_pool(name="pOut", bufs=2))

            state = {}
            pending = []

            def emit_a_geom(h):
                col0 = h * TP
                st = {}
                st["targ"] = targ = pP.tile([128, TP], F32, tag="P2", name=f"targ_{h}")
                st["n01"] = n01 = pAt.tile([128, 2 * TP], F32, tag="sA", name=f"n01_{h}")

                def comp(big, off, c, w=TQ):
                    return big[:, off : off + 3 * w].rearrange(
                        "p (t c) -> p t c", c=3
                    )[:, :, c]

                qranges = ([TQ // 2, TQ // 2, TQ, TQ] if h == 0
                           else [TQ, TQ, TQ])
                qoff = 0
                for q, qw in enumerate(qranges):
                    qs = slice(qoff, qoff + qw)
                    e0 = 3 * (col0 + qoff)
                    V01 = pV.tile([128, 2, 3 * qw], F32, tag="v", name=f"V01_{h}_{q}")
                    nc.sync.dma_start(out=V01[:], in_=v01_h[:, :, e0 : e0 + 3 * qw])
                    if not cdma[0]:
                        load_consts()
                    V01f = V01[:].rearrange("p j e -> p (j e)")

                    SQ = pSq.tile([128, 6 * qw], F32, tag="sq", name=f"SQ_{h}_{q}")
                    act(SQ[:], V01f, A.Square)

                    # both vectors' norms in one strided op pair:
                    # [128, 2, qw] views (j = which vector)
                    nq = n01[:].rearrange("p (j t) -> p j t", j=2)[:, :, qoff : qoff + qw]
                    def compj(c):
                        return SQ[:].rearrange("p (j t c) -> p j t c", j=2, c=3)[:, :, :, c]
                    nc.vector.tensor_add(nq, compj(0), compj(1))
                    nc.vector.tensor_add(nq, nq, compj(2))

                    PR = pPr.tile([128, 3 * qw], F32, tag="pr", name=f"PR_{h}_{q}")
                    nc.vector.tensor_mul(PR[:], V01f[:, : 3 * qw], V01f[:, 3 * qw :])
                    nc.vector.tensor_add(targ[:, qs], comp(PR, 0, 0, qw), comp(PR, 0, 1, qw))
                    nc.vector.tensor_add(targ[:, qs], targ[:, qs], comp(PR, 0, 2, qw))
                    qoff += qw
                return st

            def emit_ladder(h, st):
                # critical path to the C-phase gg chain: keep these DVE links
                # ahead of the next part's bulk adds in the queue
                targ = st["targ"]
                n01 = st["n01"]
                st["d01"] = d01 = pP.tile([128, 2 * TP], F32, tag="P0", name=f"d01_{h}")
                lq = pP.tile([128, TP], F32, tag="P3", name=f"lq_{h}")
                act(d01[:], n01[:], A.Ln)
                nc.vector.tensor_add(lq[:], d01[:, :TP], d01[:, TP:])
                act(lq[:], lq[:], A.Exp, scale=-0.5)
                nc.vector.tensor_mul(targ[:], targ[:], lq[:])

                st["sa"] = sa = pP.tile([128, TP], F32, tag="P4", name=f"sa_{h}")
                st["sb"] = sb = pP.tile([128, TP], F32, tag="P5", name=f"sb_{h}")
                act(sa[:], targ[:], A.Ln, scale=-0.475, bias=0.5)
                act(sa[:], sa[:], A.Exp, scale=0.5)
                act(sb[:], targ[:], A.Ln, scale=0.475, bias=0.5)
                act(sb[:], sb[:], A.Exp, scale=0.5)

            def emit_a_rest(h, st):
                d01 = st["d01"]
                act(d01[:], d01[:], A.Exp, scale=0.5)
                dm = pP.tile([128, TP], F32, tag="P6", name=f"dm_{h}")
                nc.vector.tensor_add(dm[:], d01[:, :TP], d01[:, TP:])
                st["f2"] = f2 = [
                    pG.tile([128, TP], BF16, tag=f"f2_{a}", name=f"f2_{h}_{a}")
                    for a in range(4)
                ]
                for a in range(4):
                    act(f2[a][:], dm[:], A.Square,
                        scale=se / 2.0, bias=-se * float(shfa[a]))
                    act(f2[a][:], f2[a][:], A.Exp, scale=-1.0)

            def emit_b(h, st):
                d01 = st["d01"]
                # sfc2_01 = sin^2(pi*d/7) for both vectors in one [128, 2TP] tile
                st["sfc2"] = sfc2 = pP.tile(
                    [128, 2 * TP], F32, tag="P1", name=f"sfc2_{h}"
                )
                act(sfc2[:], d01[:], A.Sin, scale=PI / 7.0)
                act(sfc2[:], sfc2[:], A.Square)

            def emit_c_s(h, st, s_lo, s_hi):
                # gg/ln/exp blocks for s in [s_lo, s_hi); finals one s behind
                sa, sb = st["sa"], st["sb"]
                f1, ggs = st["f1"], st["ggs"]
                for s in range(s_lo, s_hi):
                    c1 = math.cos(float(shfz[s]) / 2.0)
                    s1 = math.sin(float(shfz[s]) / 2.0)
                    gg = pLo.tile([128, TP], F32, tag="gg", bufs=4,
                                  name=f"gg_{h}_{s}")
                    if uniform_z and s >= 2:
                        nc.vector.scalar_tensor_tensor(
                            gg[:], ggs[s - 1][:], 2.0 * math.cos(dz2),
                            ggs[s - 2][:], op0=Op.mult, op1=Op.subtract,
                        )
                    else:
                        vv = pLo.tile([128, TP], F32, tag="vv", bufs=1,
                                      name=f"vv_{h}_{s}")
                        nc.vector.tensor_scalar_mul(vv[:], sa[:], s1)
                        nc.vector.scalar_tensor_tensor(
                            gg[:], sb[:], c1, vv[:], op0=Op.mult, op1=Op.add
                        )
                    ggs.append(gg)
                    lo = pLo.tile([128, TP], F32, tag="lo", bufs=1,
                                  name=f"lo_{h}_{s}")
                    act(lo[:], gg[:], A.Ln)
                    fs = pG.tile([128, TP], BF16, tag=f"f1_{s}", name=f"f1_{h}_{s}")
                    act(fs[:], lo[:], A.Exp, scale=2.0 * zeta)
                    f1.append(fs)
                    if s >= 1 and "g2" in st:
                        emit_finals(h, s - 1, f1[s - 1], st["g2"])
                if s_hi == 8:
                    emit_finals(h, 7, f1[7], st["g2"])

            def emit_c_head(h, st):
                sfc2, f2 = st["sfc2"], st["f2"]
                fcj2f = pP.tile([128, TP], F32, tag="P6", name=f"fcj2f_{h}")
                nc.vector.tensor_scalar(
                    fcj2f[:], sfc2[:, :TP], 2.0, 2.0, op0=Op.mult, op1=Op.subtract
                )
                fcj2 = pG.tile([128, TP], BF16, tag="fcj2", name=f"fcj2_{h}")
                nc.vector.scalar_tensor_tensor(
                    fcj2[:], sfc2[:, TP:], 1.0, fcj2f[:], op0=Op.subtract, op1=Op.mult
                )
                st["g2"] = g2 = [
                    pG.tile([128, TP], BF16, tag=f"g2_{a}", name=f"g2_{h}_{a}")
                    for a in range(4)
                ]
                for a in range(4):
                    nc.vector.tensor_mul(g2[a][:], f2[a][:], fcj2[:])
                # catch up the finals deferred while g2 didn't exist yet
                f1 = st["f1"]
                for s in range(len(f1) - 1):
                    emit_finals(h, s, f1[s], g2)

            def emit_finals(h, s, fs, g2):
                # 4 output channels (a*8+s); one s behind f1 production so the
                # DVE queue never stalls on the Exp
                col0 = h * TP
                cs = slice(col0, col0 + TP)
                ot = pOut.tile([128, 4, TP], BF16, tag="out", bufs=4,
                               name=f"ot_{h}_{s}")
                for a in range(4):
                    nc.vector.tensor_mul(ot[:, a, :], fs[:], g2[a][:])
                last = h == H - 1 and s == 7
                na = 2 if last else 4
                for a0 in range(0, 4, na):
                    nc.sync.dma_start(
                        out=out_h[:, 8 * a0 + s : 8 * (a0 + na - 1) + s + 1 : 8, cs],
                        in_=ot[:, a0 : a0 + na, :],
                    )

            # Pipelined emission: part h's ACT-heavy A runs while part h-1's
            # DVE-heavy C grinds (all ln/exp table set, so the ACT stream
            # interleaves without table switches).
            # keep the DVE queue free of ACT-blocked ops: ladder and the
            # first gg blocks go ahead of dm/f2 (rest) and the trig sines
            st0 = emit_a_geom(0)
            emit_ladder(0, st0)
            st1 = emit_a_geom(1)
            st0["f1"], st0["ggs"] = [], []
            st1["f1"], st1["ggs"] = [], []
            emit_c_s(0, st0, 0, 3)
            emit_a_rest(0, st0)
            emit_b(0, st0)
            emit_ladder(1, st1)
            emit_c_head(0, st0)
            emit_c_s(0, st0, 3, 8)
            emit_c_s(1, st1, 0, 3)
            emit_a_rest(1, st1)
            emit_b(1, st1)
            emit_c_head(1, st1)
            emit_c_s(1, st1, 3, 8)

    nc.finalize()
    _fix_act_table_loads(nc)
    return nc


def _fix_act_table_loads(nc):
    """Replace Bacc's per-function act-table loads (which thrash between
    the Ln-only and Exp-only sets) with minimal loads of sets that cover
    whole phases: natural_log_exp_and_others for Ln/Exp/Square,
    trig_and_small for Sin."""
    from concourse.hw_specs import get_activation_tables

    tables = list(get_activation_tables(nc.m.arch).items())
    name_to_id = {n: i for i, (n, _) in enumerate(tables)}
    prefer_order = ["natural_log_exp_and_others", "trig_and_small"]
    for b in nc.m.functions[0].blocks:
        insts = b.instructions
        loads = [i for i in insts if type(i).__name__ == "InstLoadActFuncSet"]
        if not loads:
            continue
        for ld in loads:
            insts.remove(ld)
        spare = list(loads)
        current_funcs = None
        ins_pts = []  # (position instruction, set name)
        for ins_ in insts:
            if isinstance(ins_, mybir.InstActivation):
                fn = ins_.func
                if current_funcs is not None and fn in current_funcs:
                    continue
                chosen = None
                for n in prefer_order:
                    if fn in dict(tables)[n]:
                        chosen = n
                        break
                if chosen is None:
                    for n, s in tables:
                        if fn in s:
                            chosen = n
                            break
                assert chosen is not None, f"no act table covers {fn}"
                ins_pts.append((ins_, chosen))
                current_funcs = dict(tables)[chosen]
        for anchor, set_name in ins_pts:
            ld = spare.pop()
            ld.act_func_set_id = name_to_id[set_name]
            insts.insert(insts.index(anchor), ld)



_BUILD_CACHE = {}


def kernel(vectors12, EtaA, Zeta, ShfA, ShfZ, _trace=False):
    global LAST_RESULT
    eta = float(np.asarray(EtaA).reshape(-1)[0])
    zeta = float(np.asarray(Zeta).reshape(-1)[0])
    shfa = [float(x) for x in np.asarray(ShfA).reshape(-1)]
    shfz = [float(x) for x in np.asarray(ShfZ).reshape(-1)]
    assert len(shfa) == 4 and len(shfz) == 8

    key = (eta, zeta, tuple(shfa), tuple(shfz))
    nc = _BUILD_CACHE.get(key)
    if nc is None:
        nc = _build(eta, zeta, shfa, shfz)
        _BUILD_CACHE[key] = nc

    v = np.asarray(vectors12, dtype=np.float32)
    assert v.shape == (2, P_TOTAL, 3)
    in_maps = []
    for i in range(N_CORES):
        shard = np.ones((2, NP_PAD, 3), dtype=np.float32)
        shard[:, :PC, :] = v[:, i * PC : (i + 1) * PC, :]
        in_maps.append({"vectors12": shard})

    res = run_bass_kernel_spmd(nc, in_maps, core_ids=list(range(N_CORES)), trace=_trace)
    LAST_RESULT = res

    full = np.empty((P_TOTAL, 32), dtype=np.float32)
    for i in range(N_CORES):
        o = res.results[i]["out"]  # (32, NP_PAD) bf16
        full[i * PC : (i + 1) * PC, :] = o[:, :PC].T.astype(np.float32)
    return full

